# revision 1
# baseline (speedup 1.0000x reference)
"""Gabor layer Trainium2 kernel.

Per gabor g and pixel (x,y) the contribution is
  amp[g,c] * exp(E_g) * cos(S_g + phase[g,c])
with E quadratic and S affine in pixel coords. Using
cos(S+p) = cos(p)cos(S) - sin(p)sin(S) the channel sum over g becomes a
matmul over gauss*cos(S) / gauss*sin(S) planes (contraction = gabors).

Host-side (float64): clip/fold all parameters, cull gabors per 64-row core
strip (a gabor whose gaussian never exceeds 1e-7 in the strip is dropped;
counts are ~70 of 256 for normal inputs, padded to a 128 chunk), build
per-tile weight tables.

Device pipeline per 512-pixel tile (16x32 px), Gc = 128 gabor rows:
  PE : E = WE^T @ feat6          (K=6 fp32, tile-centered delta features --
                                  full fp32 matmul for the cancellation)
       S = WShi^T @ onehot + WSlo^T @ onehot   (K=48 bf16 x2 passes;
             S[g,p] = A[g,row(p)] + B[g,col(p)], tables wrapped to [-pi,pi)
             and hi/lo split on host, one-hot products are exact in bf16)
  ACT: gauss = Exp(E);  t2 = Sin(S*~0.5);  u = Square(t2*sqrt2) = 2*t2^2
  DVE: w1 = add_range_wrap(S) in [-pi,pi];  (ACT: sS = Sin(w1))
       p1n = (u-1)*gauss = -cos(S)*gauss;   p2 = gauss*sS
  PE : out[3,512] += [-alpha; beta]^T @ [p1n; p2]   (2x K=128 fp32)
Tiles run in blocks of B with all Exp ops grouped before the Sin ops so the
ACT table reload (1.3us) happens twice per block, not per tile (Square/Copy
are in every ACT table set).

Sharding: 8 cores x 64-row strips, no collectives; final clamp and strip
concatenation on host.
"""

import os
import sys

import numpy as np

for _p in ("/opt/trn_rl_repo",):
    if os.path.isdir(_p) and _p not in sys.path:
        sys.path.append(_p)

H = W = 512
G = 256
NCORES = 8
SH = H // NCORES      # strip rows per core
TR, TC = 16, 32       # tile rows x cols
N = TR * TC           # 512 pixels per tile
TPR = W // TC         # tiles per strip row = 16
NT = (SH // TR) * TPR # tiles per core = 64
B = 8                 # tiles per block (ACT table phase granularity)
NBLK = NT // B
KS = TR + TC          # one-hot feature rows for the S matmul
PI = float(np.pi)
SCALE_HALF = float(np.float32(0.5 * (1.0 - 2.4e-7)))
CULL_THR = 1e-7       # drop gabors whose max contribution in-strip is below

_PROGRAMS = {}


def _build_program(nchunk):
    from concourse import bacc, mybir, tile

    f32 = mybir.dt.float32
    bf16 = mybir.dt.bfloat16
    Act = mybir.ActivationFunctionType
    Alu = mybir.AluOpType
    Gc = 128 * nchunk

    nc = bacc.Bacc("TRN2", target_bir_lowering=False, debug=False,
                   num_devices=NCORES)

    featd = nc.dram_tensor("feat", [6, NT, N], f32, kind="ExternalInput")
    wed = nc.dram_tensor("we", [6, NT, Gc], f32, kind="ExternalInput")
    wshd = nc.dram_tensor("wsh", [KS, NT, Gc], bf16, kind="ExternalInput")
    wsld = nc.dram_tensor("wsl", [KS, NT, Gc], bf16, kind="ExternalInput")
    ohd = nc.dram_tensor("onehot", [KS, N], bf16, kind="ExternalInput")
    abd = nc.dram_tensor("ab", [128, nchunk * 2 * 3], f32, kind="ExternalInput")
    outd = nc.dram_tensor("out", [3, SH, W], f32, kind="ExternalOutput")

    with tile.TileContext(nc) as tc:
        with (
            tc.tile_pool(name="io", bufs=1) as iop,
            tc.tile_pool(name="gauss", bufs=B + 2) as gp,
            tc.tile_pool(name="trig", bufs=3) as trigp,
            tc.tile_pool(name="prod", bufs=3) as pp,
            tc.tile_pool(name="mm1", bufs=4, space="PSUM") as mm1p,
            tc.tile_pool(name="acc", bufs=2, space="PSUM") as accp,
        ):
            ab_sb = iop.tile([128, nchunk * 2 * 3], f32, tag="ab")
            nc.sync.dma_start(out=ab_sb[:], in_=abd[:])
            oh_sb = iop.tile([KS, N], bf16, tag="oh")
            nc.sync.dma_start(out=oh_sb[:], in_=ohd[:])

            for blk in range(NBLK):
                t0 = blk * B
                fb = iop.tile([6, B, N], f32, tag="feat", bufs=2)
                nc.sync.dma_start(out=fb[:], in_=featd[:, t0:t0 + B, :])
                we = iop.tile([6, B, Gc], f32, tag="we", bufs=2)
                nc.sync.dma_start(out=we[:], in_=wed[:, t0:t0 + B, :])
                wsh = iop.tile([KS, B, Gc], bf16, tag="wsh", bufs=2)
                nc.sync.dma_start(out=wsh[:], in_=wshd[:, t0:t0 + B, :])
                wsl = iop.tile([KS, B, Gc], bf16, tag="wsl", bufs=2)
                nc.sync.dma_start(out=wsl[:], in_=wsld[:, t0:t0 + B, :])

                # Phase A: gaussians for the whole block (Exp table loaded).
                gts = []
                for t in range(B):
                    mE = mm1p.tile([128, nchunk, N], f32, tag="m1", name="mE")
                    for c in range(nchunk):
                        nc.tensor.matmul(
                            mE[:, c, :],
                            we[:, t, c * 128:(c + 1) * 128],
                            fb[:, t, :],
                            start=True, stop=True,
                        )
                    g = gp.tile([128, nchunk, N], f32, tag="gauss", name="gauss")
                    nc.scalar.activation(g[:], mE[:], Act.Exp)
                    gts.append(g)

                # Phase B: sinusoid + products + reduction (Sin table loaded).
                for t in range(B):
                    mS = mm1p.tile([128, nchunk, N], f32, tag="m1", name="mS")
                    for c in range(nchunk):
                        nc.tensor.matmul(
                            mS[:, c, :],
                            wsh[:, t, c * 128:(c + 1) * 128],
                            oh_sb[:],
                            start=True, stop=False,
                        )
                        nc.tensor.matmul(
                            mS[:, c, :],
                            wsl[:, t, c * 128:(c + 1) * 128],
                            oh_sb[:],
                            start=False, stop=True,
                        )
                    t2 = trigp.tile([128, nchunk, N], f32, tag="t2", name="t2")
                    nc.scalar.activation(t2[:], mS[:], Act.Sin, scale=SCALE_HALF)
                    w1 = trigp.tile([128, nchunk, N], f32, tag="w1", name="w1")
                    nc.vector.add_range_wrap(w1[:], mS[:], 0.0, PI, 2.0 * PI)
                    ss = trigp.tile([128, nchunk, N], f32, tag="ss", name="ss")
                    nc.scalar.activation(ss[:], w1[:], Act.Sin)

                    g = gts[t]
                    u = trigp.tile([128, nchunk, N], f32, tag="u", name="u")
                    # Square is in every ACT table set: no table reload.
                    nc.scalar.activation(u[:], t2[:], Act.Square,
                                         scale=float(np.sqrt(2.0)))
                    p1 = pp.tile([128, nchunk, N], f32, tag="p1", name="p1")
                    nc.vector.scalar_tensor_tensor(
                        p1[:], u[:], 1.0, g[:], Alu.subtract, Alu.mult)
                    p2 = pp.tile([128, nchunk, N], f32, tag="p2", name="p2")
                    nc.vector.tensor_mul(p2[:], g[:], ss[:])

                    if t % 2 == 0:
                        po = accp.tile([3, 2, TR, TC], f32, tag="po", name="po")
                    chunks = [(p1, c) for c in range(nchunk)] + \
                             [(p2, c) for c in range(nchunk)]
                    for ci, (src, c) in enumerate(chunks):
                        nc.tensor.matmul(
                            po[:, t % 2],
                            ab_sb[:, ci * 3:(ci + 1) * 3],
                            src[:, c, :],
                            start=(ci == 0), stop=(ci == len(chunks) - 1),
                        )
                    if t % 2 == 1:
                        ob = pp.tile([3, 2, TR, TC], f32, tag="ob", name="ob")
                        nc.scalar.copy(ob[:], po[:])
                        for h in range(2):
                            ti = t0 + t - 1 + h
                            trow, tcol = divmod(ti, TPR)
                            nc.sync.dma_start(
                                out=outd[:, trow * TR:(trow + 1) * TR,
                                         tcol * TC:(tcol + 1) * TC],
                                in_=ob[:, h],
                            )

    nc.compile()
    return nc


def _wrap(x):
    return np.mod(x + np.pi, 2.0 * np.pi) - np.pi


def _host_arrays(inputs):
    """Fold parameters, cull gabors per core, build device arrays."""
    gx = np.asarray(inputs["grid_x"], np.float64)
    gy = np.asarray(inputs["grid_y"], np.float64)
    u = np.clip(np.asarray(inputs["u"], np.float64), -1, 1)
    v = np.clip(np.asarray(inputs["v"], np.float64), -1, 1)
    th = np.clip(np.asarray(inputs["theta"], np.float64), -2, 2) * (2 * np.pi)
    sig = np.clip(np.asarray(inputs["rel_sigma"], np.float64), 0.001, 1.0)
    rf = np.clip(np.asarray(inputs["rel_freq"], np.float64), -5, 5)
    gam = np.clip(np.asarray(inputs["gamma"], np.float64), 0.0001, 1.0)
    psi = np.clip(np.asarray(inputs["psi"], np.float64), -1, 1)
    amp = np.clip(np.asarray(inputs["amplitude"], np.float64), 0, 1)

    cr, sr = np.cos(th), np.sin(th)
    cx = -(cr * u + sr * v)       # x_rot = cr*X + sr*Y + cx
    cy = sr * u - cr * v          # y_rot = -sr*X + cr*Y + cy
    p = 1.0 / (2.0 * sig * sig)
    q = 1.0 / (2.0 * gam * gam)
    freq = 2 * np.pi / np.exp(rf)
    phase = psi * (2 * np.pi)                     # [G,3]
    alpha = amp * np.cos(phase)                   # [G,3]
    beta = -amp * np.sin(phase)

    # --- cull gabors per core: the reference only evaluates at pixel
    # positions, so the keep test is the exact per-pixel max of E over the
    # strip (full resolution -- gamma can be 1e-4, a ridge 0.05 px wide).
    ampmax = amp.max(1)
    elim = np.log(np.maximum(CULL_THR / np.maximum(ampmax, 1e-30), 1e-300)) - 1.0
    keep_lists = []
    crf = cr.astype(np.float32)[:, None]
    srf = sr.astype(np.float32)[:, None]
    pf = p.astype(np.float32)[:, None]
    qf = q.astype(np.float32)[:, None]
    for core in range(NCORES):
        Xs = np.asarray(gx[core * SH:(core + 1) * SH], np.float32).ravel()[None, :]
        Ys = np.asarray(gy[core * SH:(core + 1) * SH], np.float32).ravel()[None, :]
        dx = Xs - u.astype(np.float32)[:, None]
        dy = Ys - v.astype(np.float32)[:, None]
        xr = dx * crf + dy * srf
        yr = dy * crf - dx * srf
        quad = xr * xr * pf
        quad += yr * yr * qf
        Em = -quad.min(1)
        keep = np.flatnonzero(Em >= elim)
        keep_lists.append(keep)
    gmax = max(len(k) for k in keep_lists)
    nchunk = max(1, -(-gmax // 128))
    Gc = 128 * nchunk

    # Tile grids: [total_tiles, N] with strip-row-major tile order.
    Xt = gx.reshape(H // TR, TR, W // TC, TC).transpose(0, 2, 1, 3).reshape(-1, N)
    Yt = gy.reshape(H // TR, TR, W // TC, TC).transpose(0, 2, 1, 3).reshape(-1, N)
    Xc = Xt.mean(1)
    Yc = Yt.mean(1)
    dxf = Xt - Xc[:, None]
    dyf = Yt - Yc[:, None]
    feat = np.stack([dxf, dyf, np.ones_like(dxf), dxf * dxf, dyf * dyf,
                     dxf * dyf], 0)             # [6, T, N]

    # Static one-hot features (bf16-exact).
    onehot = np.zeros((KS, N), np.float32)
    ii, jj = np.divmod(np.arange(N), TC)
    onehot[ii, np.arange(N)] = 1.0
    onehot[TR + jj, np.arange(N)] = 1.0

    yrow_all = Yt.reshape(-1, TR, TC)[:, :, 0]   # [T, TR]
    xcol_all = Xt.reshape(-1, TR, TC)[:, 0, :]   # [T, TC]

    in_maps = []
    for core in range(NCORES):
        keep = keep_lists[core]
        k = len(keep)
        sl = slice(core * NT, (core + 1) * NT)
        crk, srk = cr[keep], sr[keep]
        cxk, cyk = cx[keep], cy[keep]
        pk, qk = p[keep], q[keep]
        fk = freq[keep]

        XcT = Xc[sl][:, None]
        YcT = Yc[sl][:, None]
        cxt = XcT * crk[None, :] + YcT * srk[None, :] + cxk[None, :]  # [NT,k]
        cyt = -XcT * srk[None, :] + YcT * crk[None, :] + cyk[None, :]
        WE = np.zeros((6, NT, Gc), np.float32)
        WE[0, :, :k] = -(2 * pk * crk * cxt - 2 * qk * srk * cyt)
        WE[1, :, :k] = -(2 * pk * srk * cxt + 2 * qk * crk * cyt)
        WE[2, :, :k] = -(pk * cxt * cxt + qk * cyt * cyt)
        WE[3, :, :k] = -(pk * crk * crk + qk * srk * srk)
        WE[4, :, :k] = -(pk * srk * srk + qk * crk * crk)
        WE[5, :, :k] = -(2 * pk * crk * srk - 2 * qk * srk * crk)

        yrow = yrow_all[sl]                              # [NT, TR]
        xcol = xcol_all[sl]                              # [NT, TC]
        A = _wrap(fk[None, :, None] * srk[None, :, None]
                  * (yrow[:, None, :] - YcT[:, :, None]))            # [NT,k,TR]
        Bt = _wrap(fk[None, :, None] * crk[None, :, None]
                   * (xcol[:, None, :] - XcT[:, :, None])
                   + (fk[None, :] * cxt)[:, :, None])                # [NT,k,TC]
        WS = np.zeros((NT, Gc, KS), np.float32)
        WS[:, :k, :TR] = A
        WS[:, :k, TR:] = Bt
        WSh = _to_bf16(WS).astype(np.float32)
        WSl = WS - WSh
        WSh = WSh.transpose(2, 0, 1)                     # [KS, NT, Gc]
        WSl = np.ascontiguousarray(WSl.transpose(2, 0, 1), dtype=np.float32)

        AB = np.zeros((128, nchunk * 2 * 3), np.float32)
        al = np.zeros((Gc, 3)); bt = np.zeros((Gc, 3))
        al[:k] = alpha[keep]
        bt[:k] = beta[keep]
        for c in range(nchunk):
            AB[:, 3 * c:3 * c + 3] = -al[c * 128:(c + 1) * 128]
            off = 3 * (nchunk + c)
            AB[:, off:off + 3] = bt[c * 128:(c + 1) * 128]

        in_maps.append({
            "feat": np.ascontiguousarray(feat[:, sl, :], dtype=np.float32),
            "we": np.ascontiguousarray(WE, dtype=np.float32),
            "wsh": _to_bf16(WSh),
            "wsl": _to_bf16(WSl),
            "onehot": _to_bf16(onehot),
            "ab": AB,
        })
    return in_maps, nchunk


def _to_bf16(a):
    import ml_dtypes
    return np.ascontiguousarray(a.astype(ml_dtypes.bfloat16))


def _get_program(nchunk):
    if nchunk not in _PROGRAMS:
        _PROGRAMS[nchunk] = _build_program(nchunk)
    return _PROGRAMS[nchunk]


def kernel(**inputs):
    from concourse.bass_utils import run_bass_kernel_spmd

    in_maps, nchunk = _host_arrays(inputs)
    nc = _get_program(nchunk)
    res = run_bass_kernel_spmd(nc, in_maps, list(range(NCORES)))
    out = np.empty((3, H, W), np.float32)
    for core in range(NCORES):
        out[:, core * SH:(core + 1) * SH, :] = res.results[core]["out"]
    np.clip(out, -1.0, 1.0, out=out)
    return out



# revision 2
# speedup vs baseline: 3.7469x; 3.7469x over previous
"""Gabor layer Trainium2 kernel — packed-tile formulation.

Per gabor g and pixel (x,y): amp[g,c] * exp(E) * cos(S + phase[g,c]) with E
quadratic and S affine in pixel coords. Using cos(S+p) = cos(p)cos(S) -
sin(p)sin(S), the channel sum over g is a matmul over the plane pair
(cos(S)*gauss, sin(S)*gauss) with contraction over gabors.

Key observation: with tile-centered features on a uniform grid, the matmul
rhs (tile-local monomials for E; row/col one-hots for S) is IDENTICAL for
every 16x32 tile — all per-tile variation lives in the stationary weight
tables. So rows of one 128-partition work unit can belong to DIFFERENT
tiles: each partition row is a (gabor, tile) pair. Gabors are culled per
tile (exact per-pixel E max, thr=3e-4), tiles are bin-packed ~3-4 per
128-row "pack", and every engine pass (matmul / Exp / Sin / products /
reduce) is amortized over all tiles in the pack.

Device pipeline per pack (N=512 px):
  PE : E = WE^T @ feat6            (K=6 fp32: tile-frame monomials)
       S = WS^T @ onehot           (K=48 fp16: S[row] = A[g,row]+B[g,col])
  ACT: gauss = Exp(E)              (fp16)          [Exp-table phase]
       t2 = Sin(S*~0.5); ss = Sin(wrap(S))         [Sin-table phase]
  DVE: w = add_range_wrap(S); u = t2*t2
       p1 = (u-0.5)*gauss = -cos(S)/2*gauss;  p2 = gauss*ss
  PE : po[15*gs,512] += AB^T @ [p1;p2]   (bf16 zero-col-padded AB columns
       accumulate a whole 8-pack group into ONE psum bank)
  DVE: ob = copy(po);  1 DMA out per group.
Exp ops for all packs are grouped before all Sin ops: 2 ACT table loads
per core total. Final clamp + tile unscramble on host.

Sharding: each core owns a 64-row strip; the tile->pack map is shared
across cores (per-tile row budget = max gabor count over cores) so the
single SPMD program's baked addresses are valid on every core.
"""

import os
import sys

import numpy as np

for _p in ("/opt/trn_rl_repo",):
    if os.path.isdir(_p) and _p not in sys.path:
        sys.path.append(_p)

H = W = 512
G = 256
NCORES = 8
SH = H // NCORES      # strip rows per core = 64
TR, TC = 16, 32       # tile rows x cols
N = TR * TC           # 512 pixels per tile
TPR = W // TC         # tiles per strip row = 16
TRW = SH // TR        # tile rows per strip = 4
NT = TRW * TPR        # tiles per core = 64
KS = TR + TC          # one-hot rows for the S matmul
TS = 5                # max tiles (slots) per pack
GRP = 8               # packs per psum output group (8*15=120 rows)
PI = float(np.pi)
SCALE_HALF = float(np.float32(0.5 * (1.0 - 2.4e-7)))
CULL_THR = 3e-4

_PROGRAMS = {}


def _build_program(npack, groups):
    from concourse import bacc, mybir, tile

    f32 = mybir.dt.float32
    f16 = mybir.dt.float16
    Act = mybir.ActivationFunctionType
    Alu = mybir.AluOpType

    nc = bacc.Bacc("TRN2", target_bir_lowering=False, debug=False,
                   num_devices=NCORES)

    featd = nc.dram_tensor("feat", [6, N], f32, kind="ExternalInput")
    ohd = nc.dram_tensor("onehot", [KS, N], f16, kind="ExternalInput")
    wed = nc.dram_tensor("we", [6, npack, 128], f32, kind="ExternalInput")
    wsd = nc.dram_tensor("ws", [KS, npack, 128], f16, kind="ExternalInput")
    abd = nc.dram_tensor("ab", [128, npack, 2, 3 * TS * GRP], f16,
                         kind="ExternalInput")
    ngrp = len(groups)
    outd = nc.dram_tensor("out", [ngrp, 3 * TS * GRP, N], f32,
                          kind="ExternalOutput")

    with tile.TileContext(nc) as tc:
        with (
            tc.tile_pool(name="io", bufs=1) as iop,
            tc.tile_pool(name="trig", bufs=3) as trigp,
            tc.tile_pool(name="prod", bufs=3) as pp,
            tc.tile_pool(name="m2", bufs=3, space="PSUM") as m2p,
            tc.tile_pool(name="po", bufs=2, space="PSUM") as pop,
        ):
            feat_sb = iop.tile([6, N], f32, tag="feat")
            nc.sync.dma_start(out=feat_sb[:], in_=featd[:])
            oh_sb = iop.tile([KS, N], f16, tag="oh")
            nc.sync.dma_start(out=oh_sb[:], in_=ohd[:])
            we_sb = iop.tile([6, npack, 128], f32, tag="we")
            nc.sync.dma_start(out=we_sb[:], in_=wed[:])
            ws_sb = iop.tile([KS, npack, 128], f16, tag="ws")
            nc.sync.dma_start(out=ws_sb[:], in_=wsd[:])
            ab_sb = iop.tile([128, npack, 2, 3 * TS * GRP], f16, tag="ab")
            nc.sync.dma_start(out=ab_sb[:], in_=abd[:])
            gauss = iop.tile([128, npack, N], f16, tag="gauss")

            # Phase E: all gaussians (Exp table loaded once).
            for sp in range(npack // 2):
                mE = m2p.tile([128, 2, N], f32, tag="m2", name="mE")
                for h in range(2):
                    nc.tensor.matmul(mE[:, h], we_sb[:, 2 * sp + h], feat_sb[:],
                                     start=True, stop=True)
                nc.scalar.activation(gauss[:, 2 * sp:2 * sp + 2], mE[:], Act.Exp)

            # Phase S: sinusoids + products + reduction (Sin table loaded).
            for g0, gs in groups:
                po = pop.tile([3 * TS * gs, N], f32, tag="po", name="po")
                for i in range(0, gs, 2):
                    mS = m2p.tile([128, 2, N], f32, tag="m2", name="mS")
                    for h in range(2):
                        nc.tensor.matmul(mS[:, h], ws_sb[:, g0 + i + h],
                                         oh_sb[:], start=True, stop=True)
                    t2 = trigp.tile([128, 2, N], f16, tag="t2", name="t2")
                    nc.scalar.activation(t2[:], mS[:], Act.Sin,
                                         scale=SCALE_HALF)
                    w = trigp.tile([128, 2, N], f16, tag="w", name="w")
                    nc.vector.add_range_wrap(w[:], mS[:], 0.0, PI, 2.0 * PI)
                    ss = trigp.tile([128, 2, N], f16, tag="ss", name="ss")
                    nc.scalar.activation(ss[:], w[:], Act.Sin)
                    uu = trigp.tile([128, 2, N], f16, tag="u", name="u")
                    nc.vector.tensor_mul(uu[:], t2[:], t2[:])
                    gsl = gauss[:, g0 + i:g0 + i + 2]
                    p1 = pp.tile([128, 2, N], f16, tag="p1", name="p1")
                    nc.vector.scalar_tensor_tensor(
                        p1[:], uu[:], 0.5, gsl, Alu.subtract, Alu.mult)
                    p2 = pp.tile([128, 2, N], f16, tag="p2", name="p2")
                    nc.vector.tensor_mul(p2[:], gsl, ss[:])
                    for h in range(2):
                        pk = g0 + i + h
                        nc.tensor.matmul(
                            po[:], ab_sb[:, pk, 0, :3 * TS * gs], p1[:, h],
                            start=(i + h == 0), stop=False)
                        nc.tensor.matmul(
                            po[:], ab_sb[:, pk, 1, :3 * TS * gs], p2[:, h],
                            start=False, stop=(i + h == gs - 1))
                ob = pp.tile([3 * TS * gs, N], f32, tag="ob", name="ob")
                nc.vector.tensor_scalar_add(ob[:], po[:], 0.0)
                grp = g0 // GRP
                nc.sync.dma_start(out=outd[grp, :3 * TS * gs, :], in_=ob[:])

    nc.compile()
    return nc


def _wrap(x):
    return np.mod(x + np.pi, 2.0 * np.pi) - np.pi


def _pack_tiles(mk):
    """First-fit-decreasing bin packing of tiles into 128-row packs with at
    most TS tiles each. mk[t] = padded row budget of tile t (shared across
    cores). Returns list of packs, each a list of (tile, row_offset)."""
    order = np.argsort(-mk, kind="stable")
    packs = []      # (rows_used, [(t, off)])
    for t in order:
        need = int(mk[t])
        placed = False
        for p in packs:
            if p[0] + need <= 128 and len(p[1]) < TS:
                p[1].append((int(t), p[0]))
                p[0] += need
                placed = True
                break
        if not placed:
            packs.append([need, [(int(t), 0)]])
    return [p[1] for p in packs]


def _host_arrays(inputs):
    gx = np.asarray(inputs["grid_x"], np.float64)
    gy = np.asarray(inputs["grid_y"], np.float64)
    u = np.clip(np.asarray(inputs["u"], np.float64), -1, 1)
    v = np.clip(np.asarray(inputs["v"], np.float64), -1, 1)
    th = np.clip(np.asarray(inputs["theta"], np.float64), -2, 2) * (2 * np.pi)
    sig = np.clip(np.asarray(inputs["rel_sigma"], np.float64), 0.001, 1.0)
    rf = np.clip(np.asarray(inputs["rel_freq"], np.float64), -5, 5)
    gam = np.clip(np.asarray(inputs["gamma"], np.float64), 0.0001, 1.0)
    psi = np.clip(np.asarray(inputs["psi"], np.float64), -1, 1)
    amp = np.clip(np.asarray(inputs["amplitude"], np.float64), 0, 1)

    cr, sr = np.cos(th), np.sin(th)
    cx = -(cr * u + sr * v)       # x_rot = cr*X + sr*Y + cx
    cy = sr * u - cr * v
    p = 1.0 / (2.0 * sig * sig)
    q = 1.0 / (2.0 * gam * gam)
    freq = 2 * np.pi / np.exp(rf)
    phase = psi * (2 * np.pi)
    alpha = amp * np.cos(phase)                   # [G,3]
    beta = -amp * np.sin(phase)

    ampmax = amp.max(1)
    elim = np.log(np.maximum(CULL_THR / np.maximum(ampmax, 1e-30), 1e-300))

    # --- per (core, tile) gabor culling: exact per-pixel E max over tile.
    crf = cr.astype(np.float32)[:, None, None]
    srf = sr.astype(np.float32)[:, None, None]
    pf = p.astype(np.float32)[:, None, None]
    qf = q.astype(np.float32)[:, None, None]
    keeps = []                     # keeps[core][t] = gabor index array
    for core in range(NCORES):
        Xs = np.asarray(gx[core * SH:(core + 1) * SH], np.float32)
        Ys = np.asarray(gy[core * SH:(core + 1) * SH], np.float32)
        dx = Xs[None] - u.astype(np.float32)[:, None, None]
        dy = Ys[None] - v.astype(np.float32)[:, None, None]
        xr = dx * crf + dy * srf
        yr = dy * crf - dx * srf
        E = -(xr * xr * pf + yr * yr * qf)
        Em = E.reshape(G, TRW, TR, TPR, TC).max(axis=(2, 4))   # [G,4,16]
        keeps.append([np.flatnonzero(Em[:, t // TPR, t % TPR] >= elim)
                      for t in range(NT)])

    kmat = np.array([[len(keeps[c][t]) for t in range(NT)]
                     for c in range(NCORES)])
    mk = np.maximum(kmat.max(axis=0), 1)           # shared row budget
    packs = _pack_tiles(mk)
    npack = len(packs)
    if npack % 2:
        packs.append([])
        npack += 1
    groups = []                                     # (first_pack, size)
    g0 = 0
    while g0 < npack:
        groups.append((g0, min(GRP, npack - g0)))
        g0 += GRP
    assert all(gs % 2 == 0 for _, gs in groups)

    # canonical tile-local coordinate patterns (identical for every tile)
    xs = gx[0]
    ys = gy[:, 0]
    dxf = np.tile(xs[:TC] - xs[:TC].mean(), TR)            # [N]
    dyf = np.repeat(ys[:TR] - ys[:TR].mean(), TC)          # [N]
    feat = np.stack([dxf, dyf, np.ones_like(dxf), dxf * dxf, dyf * dyf,
                     dxf * dyf], 0).astype(np.float32)
    onehot = np.zeros((KS, N), np.float16)
    ii, jj = np.divmod(np.arange(N), TC)
    onehot[ii, np.arange(N)] = 1.0
    onehot[TR + jj, np.arange(N)] = 1.0
    Xc_col = xs.reshape(TPR, TC).mean(1)                   # per tile-col
    Yc_row = ys.reshape(H // TR, TR).mean(1)               # per global tile-row
    yoff = ys[:TR] - ys[:TR].mean()                        # [TR]
    xoff = xs[:TC] - xs[:TC].mean()                        # [TC]

    # map (pack, slot) -> tile and po-row base; shared across cores
    tile_map = []                  # (grp, row_base, tile)
    for pi, pk in enumerate(packs):
        grp = pi // GRP
        ib = (pi % GRP) * 3 * TS
        for s, (t, off) in enumerate(pk):
            tile_map.append((grp, ib + 3 * s, t))

    in_maps = []
    for core in range(NCORES):
        WE = np.zeros((6, npack, 128), np.float32)
        WS = np.zeros((KS, npack, 128), np.float16)
        AB = np.zeros((128, npack, 2, 3 * TS * GRP), np.float16)
        for pi, pk in enumerate(packs):
            ib = (pi % GRP) * 3 * TS
            for s, (t, off) in enumerate(pk):
                g_ids = keeps[core][t]
                k = len(g_ids)
                if k == 0:
                    continue
                trow, tcol = divmod(t, TPR)
                Xc = Xc_col[tcol]
                Yc = Yc_row[core * TRW + trow]
                crk, srk = cr[g_ids], sr[g_ids]
                pk_, qk = p[g_ids], q[g_ids]
                cxt = Xc * crk + Yc * srk + cx[g_ids]
                cyt = -Xc * srk + Yc * crk + cy[g_ids]
                rows = slice(off, off + k)
                WE[0, pi, rows] = -(2 * pk_ * crk * cxt - 2 * qk * srk * cyt)
                WE[1, pi, rows] = -(2 * pk_ * srk * cxt + 2 * qk * crk * cyt)
                WE[2, pi, rows] = -(pk_ * cxt * cxt + qk * cyt * cyt)
                WE[3, pi, rows] = -(pk_ * crk * crk + qk * srk * srk)
                WE[4, pi, rows] = -(pk_ * srk * srk + qk * crk * crk)
                WE[5, pi, rows] = -2 * crk * srk * (pk_ - qk)
                fk = freq[g_ids]
                A = _wrap(fk[:, None] * srk[:, None] * yoff[None, :])
                Bt = _wrap(fk[:, None] * crk[:, None] * xoff[None, :]
                           + (fk * cxt)[:, None])
                WS[:TR, pi, rows] = A.T
                WS[TR:, pi, rows] = Bt.T
                for ch in range(3):
                    AB[rows, pi, 0, ib + 3 * s + ch] = -2 * alpha[g_ids, ch]
                    AB[rows, pi, 1, ib + 3 * s + ch] = beta[g_ids, ch]
        in_maps.append({
            "feat": feat, "onehot": onehot,
            "we": WE,
            "ws": np.ascontiguousarray(WS),
            "ab": np.ascontiguousarray(AB),
        })
    return in_maps, npack, tuple(groups), tile_map


def _get_program(npack, groups):
    key = (npack, groups)
    if key not in _PROGRAMS:
        _PROGRAMS[key] = _build_program(npack, groups)
    return _PROGRAMS[key]


def kernel(**inputs):
    from concourse.bass_utils import run_bass_kernel_spmd

    in_maps, npack, groups, tile_map = _host_arrays(inputs)
    nc = _get_program(npack, groups)
    res = run_bass_kernel_spmd(nc, in_maps, list(range(NCORES)))
    out = np.empty((3, H, W), np.float32)
    for core in range(NCORES):
        o = res.results[core]["out"]               # [ngrp, 120, N]
        for grp, rb, t in tile_map:
            trow, tcol = divmod(t, TPR)
            out[:, core * SH + trow * TR:core * SH + (trow + 1) * TR,
                tcol * TC:(tcol + 1) * TC] = \
                o[grp, rb:rb + 3].reshape(3, TR, TC)
    np.clip(out, -1.0, 1.0, out=out)
    return out


# revision 4
# speedup vs baseline: 4.6137x; 1.2313x over previous
"""Gabor layer Trainium2 kernel — packed-tile formulation.

Per gabor g and pixel (x,y): amp[g,c] * exp(E) * cos(S + phase[g,c]) with E
quadratic and S affine in pixel coords. Using cos(S+p) = cos(p)cos(S) -
sin(p)sin(S), the channel sum over g is a matmul over the plane pair
(cos(S)*gauss, sin(S)*gauss) with contraction over gabors.

Key observation: with tile-centered features on a uniform grid, the matmul
rhs (tile-local monomials for E; row/col one-hots for S) is IDENTICAL for
every 16x32 tile — all per-tile variation lives in the stationary weight
tables. So rows of one 128-partition work unit can belong to DIFFERENT
tiles: each partition row is a (gabor, tile) pair. Gabors are culled per
tile (exact per-pixel E max, thr=1e-3), tiles are bin-packed ~3-4 per
128-row "pack", and every engine pass (matmul / Exp / Sin / products /
reduce) is amortized over all tiles in the pack.

Device pipeline per pack (N=512 px):
  Phase A (Exp table):
    PE : E = WE^T @ feat6          (K=6 fp32r: tile-frame monomials)
         S = WS^T @ onehot         (K=48 fp16: S[px] = A[g,row]+B[g,col])
    ACT: gauss = Exp(E)  (fp16)
    DVE: w = add_range_wrap(S) in [-pi,pi)  (fp16; w == S mod 2pi)
  Phase B (Sin table):
    ACT: t2 = Sin(w*0.5)  (sin^2(w/2) == sin^2(S/2));  ss = Sin(w)
    DVE: u = t2*t2;  p2 = gauss*ss
    GPS: p1 = (u-0.5)*gauss = -cos(S)/2*gauss
    PE : po[15*gs,512] += AB^T @ [p1;p2]  (fp16 zero-col-padded AB columns
         accumulate a whole 8-pack group into ONE psum bank)
    DVE: ob = copy(po);  1 DMA out per group.
All Exp ops precede all Sin ops on the ACT queue: 2 table loads per core.
Final clamp + tile unscramble on host.

Sharding: each core owns a 64-row strip; the tile->pack map is shared
across cores (per-tile row budget = max gabor count over cores) so the
single SPMD program's baked addresses are valid on every core.
"""

import os
import sys

import numpy as np

for _p in ("/opt/trn_rl_repo",):
    if os.path.isdir(_p) and _p not in sys.path:
        sys.path.append(_p)

H = W = 512
G = 256
NCORES = 8
SH = H // NCORES      # strip rows per core = 64
TR, TC = 16, 32       # tile rows x cols
N = TR * TC           # 512 pixels per tile
TPR = W // TC         # tiles per strip row = 16
TRW = SH // TR        # tile rows per strip = 4
NT = TRW * TPR        # tiles per core = 64
KS = TR + TC          # one-hot rows for the S matmul
TS = 5                # max tiles (slots) per pack
GRP = 8               # packs per psum output group (8*15=120 rows)
PI = float(np.pi)
CULL_THR = 1e-3
USE_F32R = True

_PROGRAMS = {}


def _build_program(npack, groups):
    from concourse import bacc, mybir, tile

    f32 = mybir.dt.float32
    f32r = mybir.dt.float32r if USE_F32R else f32
    f16 = mybir.dt.float16
    Act = mybir.ActivationFunctionType
    Alu = mybir.AluOpType

    nc = bacc.Bacc("TRN2", target_bir_lowering=False, debug=False,
                   num_devices=NCORES)

    featd = nc.dram_tensor("feat", [6, N], f32r, kind="ExternalInput")
    ohd = nc.dram_tensor("onehot", [KS, N], f16, kind="ExternalInput")
    wed = nc.dram_tensor("we", [6, npack, 128], f32r, kind="ExternalInput")
    wsd = nc.dram_tensor("ws", [KS, npack, 128], f16, kind="ExternalInput")
    abd = nc.dram_tensor("ab", [128, npack, 2, 3 * TS * GRP], f16,
                         kind="ExternalInput")
    ngrp = len(groups)
    outd = nc.dram_tensor("out", [ngrp, 3 * TS * GRP, N], f32,
                          kind="ExternalOutput")

    with tile.TileContext(nc) as tc:
        with (
            tc.tile_pool(name="io", bufs=1) as iop,
            tc.tile_pool(name="trig", bufs=3) as trigp,
            tc.tile_pool(name="prod", bufs=3) as pp,
            tc.tile_pool(name="m2", bufs=3, space="PSUM") as m2p,
            tc.tile_pool(name="po", bufs=2, space="PSUM") as pop,
        ):
            feat_sb = iop.tile([6, N], f32r, tag="feat")
            nc.sync.dma_start(out=feat_sb[:], in_=featd[:])
            oh_sb = iop.tile([KS, N], f16, tag="oh")
            nc.sync.dma_start(out=oh_sb[:], in_=ohd[:])
            we_sb = iop.tile([6, npack, 128], f32r, tag="we")
            nc.sync.dma_start(out=we_sb[:], in_=wed[:])
            ws_sb = iop.tile([KS, npack, 128], f16, tag="ws")
            nc.sync.dma_start(out=ws_sb[:], in_=wsd[:])
            ab_sb = iop.tile([128, npack, 2, 3 * TS * GRP], f16, tag="ab")
            nc.sync.dma_start(out=ab_sb[:], in_=abd[:])
            gauss = iop.tile([128, npack, N], f16, tag="gauss")
            wall = iop.tile([128, npack, N], f16, tag="wall")

            # Phase A: gaussians (Exp table) + wrapped sinusoid arguments.
            for sp in range(npack // 2):
                mE = m2p.tile([128, 2, N], f32, tag="m2", name="mE")
                for h in range(2):
                    nc.tensor.matmul(mE[:, h], we_sb[:, 2 * sp + h], feat_sb[:],
                                     start=True, stop=True)
                nc.scalar.activation(gauss[:, 2 * sp:2 * sp + 2], mE[:], Act.Exp)
                mS = m2p.tile([128, 2, N], f32, tag="m2", name="mS")
                for h in range(2):
                    nc.tensor.matmul(mS[:, h], ws_sb[:, 2 * sp + h],
                                     oh_sb[:], start=True, stop=True)
                nc.vector.add_range_wrap(wall[:, 2 * sp:2 * sp + 2], mS[:],
                                         0.0, PI, 2.0 * PI)

            # Phase B: sinusoids + products + reduction (Sin table).
            for g0, gs in groups:
                po = pop.tile([3 * TS * gs, N], f32, tag="po", name="po")
                for i in range(0, gs, 2):
                    wsl = wall[:, g0 + i:g0 + i + 2]
                    gsl = gauss[:, g0 + i:g0 + i + 2]
                    t2 = trigp.tile([128, 2, N], f16, tag="t2", name="t2")
                    nc.scalar.activation(t2[:], wsl, Act.Sin, scale=0.5)
                    ss = trigp.tile([128, 2, N], f16, tag="ss", name="ss")
                    nc.scalar.activation(ss[:], wsl, Act.Sin)
                    uu = trigp.tile([128, 2, N], f16, tag="u", name="u")
                    nc.vector.tensor_mul(uu[:], t2[:], t2[:])
                    p1 = pp.tile([128, 2, N], f16, tag="p1", name="p1")
                    nc.vector.scalar_tensor_tensor(
                        p1[:], uu[:], 0.5, gsl, Alu.subtract, Alu.mult)
                    p2 = pp.tile([128, 2, N], f16, tag="p2", name="p2")
                    nc.gpsimd.tensor_mul(p2[:], gsl, ss[:])
                    for h in range(2):
                        pk = g0 + i + h
                        nc.tensor.matmul(
                            po[:], ab_sb[:, pk, 0, :3 * TS * gs], p1[:, h],
                            start=(i + h == 0), stop=False)
                        nc.tensor.matmul(
                            po[:], ab_sb[:, pk, 1, :3 * TS * gs], p2[:, h],
                            start=False, stop=(i + h == gs - 1))
                ob = pp.tile([3 * TS * gs, N], f32, tag="ob", name="ob")
                nc.vector.tensor_scalar_add(ob[:], po[:], 0.0)
                grp = g0 // GRP
                nc.sync.dma_start(out=outd[grp, :3 * TS * gs, :], in_=ob[:])

    nc.compile()
    return nc


def _wrap(x):
    return np.mod(x + np.pi, 2.0 * np.pi) - np.pi


def _pack_tiles(mk):
    """First-fit-decreasing bin packing of tiles into 128-row packs with at
    most TS tiles each. mk[t] = row budget of tile t (shared across cores).
    Returns list of packs, each a list of (tile, row_offset)."""
    order = np.argsort(-mk, kind="stable")
    packs = []      # [rows_used, [(t, off)]]
    for t in order:
        need = int(mk[t])
        placed = False
        for p in packs:
            if p[0] + need <= 128 and len(p[1]) < TS:
                p[1].append((int(t), p[0]))
                p[0] += need
                placed = True
                break
        if not placed:
            packs.append([need, [(int(t), 0)]])
    return [p[1] for p in packs]


def _host_arrays(inputs):
    gx = np.asarray(inputs["grid_x"], np.float64)
    gy = np.asarray(inputs["grid_y"], np.float64)
    u = np.clip(np.asarray(inputs["u"], np.float64), -1, 1)
    v = np.clip(np.asarray(inputs["v"], np.float64), -1, 1)
    th = np.clip(np.asarray(inputs["theta"], np.float64), -2, 2) * (2 * np.pi)
    sig = np.clip(np.asarray(inputs["rel_sigma"], np.float64), 0.001, 1.0)
    rf = np.clip(np.asarray(inputs["rel_freq"], np.float64), -5, 5)
    gam = np.clip(np.asarray(inputs["gamma"], np.float64), 0.0001, 1.0)
    psi = np.clip(np.asarray(inputs["psi"], np.float64), -1, 1)
    amp = np.clip(np.asarray(inputs["amplitude"], np.float64), 0, 1)

    cr, sr = np.cos(th), np.sin(th)
    cx = -(cr * u + sr * v)       # x_rot = cr*X + sr*Y + cx
    cy = sr * u - cr * v
    p = 1.0 / (2.0 * sig * sig)
    q = 1.0 / (2.0 * gam * gam)
    freq = 2 * np.pi / np.exp(rf)
    phase = psi * (2 * np.pi)
    alpha = amp * np.cos(phase)                   # [G,3]
    beta = -amp * np.sin(phase)

    ampmax = amp.max(1)
    elim = np.log(np.maximum(CULL_THR / np.maximum(ampmax, 1e-30), 1e-300))

    # --- per (core, tile) gabor culling: exact per-pixel E max over tile.
    crf = cr.astype(np.float32)[:, None, None]
    srf = sr.astype(np.float32)[:, None, None]
    pf = p.astype(np.float32)[:, None, None]
    qf = q.astype(np.float32)[:, None, None]
    keeps = []                     # keeps[core][t] = gabor index array
    for core in range(NCORES):
        Xs = np.asarray(gx[core * SH:(core + 1) * SH], np.float32)
        Ys = np.asarray(gy[core * SH:(core + 1) * SH], np.float32)
        dx = Xs[None] - u.astype(np.float32)[:, None, None]
        dy = Ys[None] - v.astype(np.float32)[:, None, None]
        xr = dx * crf + dy * srf
        yr = dy * crf - dx * srf
        E = -(xr * xr * pf + yr * yr * qf)
        Em = E.reshape(G, TRW, TR, TPR, TC).max(axis=(2, 4))   # [G,4,16]
        keeps.append([np.flatnonzero(Em[:, t // TPR, t % TPR] >= elim)
                      for t in range(NT)])

    kmat = np.array([[len(keeps[c][t]) for t in range(NT)]
                     for c in range(NCORES)])
    mk = np.maximum(kmat.max(axis=0), 1)           # shared row budget
    packs = _pack_tiles(mk)
    npack = len(packs)
    if npack % 2:
        packs.append([])
        npack += 1
    groups = []                                     # (first_pack, size)
    g0 = 0
    while g0 < npack:
        groups.append((g0, min(GRP, npack - g0)))
        g0 += GRP
    assert all(gs % 2 == 0 for _, gs in groups)

    # canonical tile-local coordinate patterns (identical for every tile)
    xs = gx[0]
    ys = gy[:, 0]
    dxf = np.tile(xs[:TC] - xs[:TC].mean(), TR)            # [N]
    dyf = np.repeat(ys[:TR] - ys[:TR].mean(), TC)          # [N]
    feat = np.stack([dxf, dyf, np.ones_like(dxf), dxf * dxf, dyf * dyf,
                     dxf * dyf], 0).astype(np.float32)
    onehot = np.zeros((KS, N), np.float16)
    ii, jj = np.divmod(np.arange(N), TC)
    onehot[ii, np.arange(N)] = 1.0
    onehot[TR + jj, np.arange(N)] = 1.0
    Xc_col = xs.reshape(TPR, TC).mean(1)                   # per tile-col
    Yc_row = ys.reshape(H // TR, TR).mean(1)               # per global tile-row
    yoff = ys[:TR] - ys[:TR].mean()                        # [TR]
    xoff = xs[:TC] - xs[:TC].mean()                        # [TC]

    # map (pack, slot) -> tile and po-row base; shared across cores
    tile_map = []                  # (grp, row_base, tile)
    for pi, pk in enumerate(packs):
        grp = pi // GRP
        ib = (pi % GRP) * 3 * TS
        for s, (t, off) in enumerate(pk):
            tile_map.append((grp, ib + 3 * s, t))

    in_maps = []
    for core in range(NCORES):
        WE = np.zeros((6, npack, 128), np.float32)
        WS = np.zeros((KS, npack, 128), np.float16)
        AB = np.zeros((128, npack, 2, 3 * TS * GRP), np.float16)
        for pi, pk in enumerate(packs):
            ib = (pi % GRP) * 3 * TS
            for s, (t, off) in enumerate(pk):
                g_ids = keeps[core][t]
                k = len(g_ids)
                if k == 0:
                    continue
                trow, tcol = divmod(t, TPR)
                Xc = Xc_col[tcol]
                Yc = Yc_row[core * TRW + trow]
                crk, srk = cr[g_ids], sr[g_ids]
                pk_, qk = p[g_ids], q[g_ids]
                cxt = Xc * crk + Yc * srk + cx[g_ids]
                cyt = -Xc * srk + Yc * crk + cy[g_ids]
                rows = slice(off, off + k)
                WE[0, pi, rows] = -(2 * pk_ * crk * cxt - 2 * qk * srk * cyt)
                WE[1, pi, rows] = -(2 * pk_ * srk * cxt + 2 * qk * crk * cyt)
                WE[2, pi, rows] = -(pk_ * cxt * cxt + qk * cyt * cyt)
                WE[3, pi, rows] = -(pk_ * crk * crk + qk * srk * srk)
                WE[4, pi, rows] = -(pk_ * srk * srk + qk * crk * crk)
                WE[5, pi, rows] = -2 * crk * srk * (pk_ - qk)
                fk = freq[g_ids]
                A = _wrap(fk[:, None] * srk[:, None] * yoff[None, :])
                Bt = _wrap(fk[:, None] * crk[:, None] * xoff[None, :]
                           + (fk * cxt)[:, None])
                WS[:TR, pi, rows] = A.T
                WS[TR:, pi, rows] = Bt.T
                for ch in range(3):
                    AB[rows, pi, 0, ib + 3 * s + ch] = -2 * alpha[g_ids, ch]
                    AB[rows, pi, 1, ib + 3 * s + ch] = beta[g_ids, ch]
        in_maps.append({
            "feat": feat, "onehot": onehot,
            "we": WE,
            "ws": np.ascontiguousarray(WS),
            "ab": np.ascontiguousarray(AB),
        })
    return in_maps, npack, tuple(groups), tile_map


def _get_program(npack, groups):
    key = (npack, groups)
    if key not in _PROGRAMS:
        _PROGRAMS[key] = _build_program(npack, groups)
    return _PROGRAMS[key]


def kernel(**inputs):
    from concourse.bass_utils import run_bass_kernel_spmd

    in_maps, npack, groups, tile_map = _host_arrays(inputs)
    nc = _get_program(npack, groups)
    res = run_bass_kernel_spmd(nc, in_maps, list(range(NCORES)))
    out = np.empty((3, H, W), np.float32)
    for core in range(NCORES):
        o = res.results[core]["out"]               # [ngrp, 120, N]
        for grp, rb, t in tile_map:
            trow, tcol = divmod(t, TPR)
            out[:, core * SH + trow * TR:core * SH + (trow + 1) * TR,
                tcol * TC:(tcol + 1) * TC] = \
                o[grp, rb:rb + 3].reshape(3, TR, TC)
    np.clip(out, -1.0, 1.0, out=out)
    return out
